# revision 25
# baseline (speedup 1.0000x reference)
"""Inverse DTCWT (biort bandpass) level-1 reconstruction as a Bass/Tile kernel.

Math: the reference is
    y = (A0 @ Yl + A1 @ lh) @ A0^T + (A0 @ hl) @ A1^T + (A2 @ hh) @ A2^T
where A* are 256x256 banded matrices (1D taps + symmetric padding folded in)
and lh/hl/hh are the c2q quad-interleaves of subband pairs (0,5)/(2,3)/(1,4).

Row r of a c2q image comes from `top` (r even) or `bot` (r odd), each a
128x256 column-interleaved image; the column interleave is expressed in the
matmul stationary access pattern (w outer, ri inner) so the DVE builds the
four 128x128 combination blocks with plain contiguous adds/subs:
    top = [w1r+w2r | w1i+w2i]   bot = [w1i-w2i | w2r-w1r]
The row interleave never materializes: contraction over rows splits into
even/odd with host-precomputed matrices Re = A^T[0::2]/sqrt2, Ro = A^T[1::2]/sqrt2.

Stage A (col filters) runs with the *image tiles stationary* producing
transposed intermediates Z[c, h] in PSUM; stage B (row filters) consumes Z
slices as stationary against A^T and accumulates all three paths into one
PSUM bank in natural orientation. No transposes anywhere.

Everything runs in bfloat16 (I/O, matmul operands; PSUM accumulates fp32).
The A* matrices are banded (13/19/13 taps), so contraction chunks only
produce a ~136-144 wide band of output columns; matmul cost is (moving
free size) cycles, so every Yl / stage-B matmul streams only its band
instead of the full 256 columns (~25% fewer PE cycles).  PSUM start=True
marks the whole 2KB bank pending-zero, so partial-width accumulation is
exact as long as each byte's first touch comes from a uniformly-pending
write (write order below guarantees this).

Sharding: pure data parallel, batch dim (8) across 8 cores.
"""
import sys

if "/opt/trn_rl_repo" not in sys.path:
    sys.path.insert(0, "/opt/trn_rl_repo")

import ml_dtypes
import numpy as np

_C, _H = 64, 256  # channels per core, image size
_NCORES = 8
_G = 4  # images (channels) per group

BF = ml_dtypes.bfloat16


def _band_matrix(h, N):
    """A @ x == colfilter(x, h) with symmetric padding, in float64."""
    h = np.asarray(h, dtype=np.float64)
    L = h.shape[0]
    m = L // 2
    A = np.zeros((N, N), dtype=np.float64)
    for i in range(N):
        for k in range(L):
            s = i + k - m
            if s < 0:
                s = -1 - s
            elif s >= N:
                s = 2 * N - 1 - s
            A[i, s] += h[L - 1 - k]
    return A


def build_consts(g0o, g1o, g2o):
    """Host-side constant tensors handed to every core."""
    A0 = _band_matrix(g0o, _H).T  # stored transposed: [r, h]
    A1 = _band_matrix(g1o, _H).T
    A2 = _band_matrix(g2o, _H).T
    s2 = np.sqrt(2.0)

    def tile2(AT):  # [256, 256] -> [128, 2, 256] with [p, kr, h] = AT[128*kr+p, h]
        return np.ascontiguousarray(
            AT.reshape(2, 128, 256).transpose(1, 0, 2)
        ).astype(BF)

    a0t, a1t, a2t = tile2(A0), tile2(A1), tile2(A2)
    # rmats[q, e/o]: per-pair col-filter matrices; pair q uses bands (q, 5-q):
    #   q=0 (lh)   -> col filter A1 ; q=1 (hh) -> A2 ; q=2 (hl) -> A0
    rmats = np.stack(
        [
            np.stack([A1[0::2] / s2, A1[1::2] / s2]),
            np.stack([A2[0::2] / s2, A2[1::2] / s2]),
            np.stack([A0[0::2] / s2, A0[1::2] / s2]),
        ]
    )  # [3, 2, 128, 256]
    rm = np.ascontiguousarray(rmats.transpose(2, 0, 1, 3)).astype(BF)  # [128,3,2,256]
    return {"a0t": a0t, "a1t": a1t, "a2t": a2t, "rmats": rm}


def build_nc(n_images):
    import concourse.bacc as bacc
    import concourse.mybir as mybir
    from concourse.tile import TileContext

    f32 = mybir.dt.float32
    bf16 = mybir.dt.bfloat16
    nc = bacc.Bacc(None, target_bir_lowering=False, debug=False)

    n_groups = n_images // _G
    yl_d = nc.declare_dram_parameter(
        "ylp", [n_groups, 128, _G, 2, 256], bf16, isOutput=False
    )
    # yhp holds two host-interleaved variants of each band, with bands in
    # pair order [0,1,2,5,4,3] so slot j pairs with slot 3+j:
    #   v=0: S_b[hr, 2w+ri] = (r|i)      -> top = S_q + S_{5-q}
    #   v=1: B_b[hr, 2w+ri] = (i|-r)     -> bot = B_q - B_{5-q}
    # Both c2q combinations are then single fully-contiguous adds/subs.
    yh_d = nc.declare_dram_parameter(
        "yhp", [n_groups, 128, 2, _G, 6, 128, 2], bf16, isOutput=False
    )
    a0t_d = nc.declare_dram_parameter("a0t", [128, 2, 256], bf16, isOutput=False)
    a1t_d = nc.declare_dram_parameter("a1t", [128, 2, 256], bf16, isOutput=False)
    a2t_d = nc.declare_dram_parameter("a2t", [128, 2, 256], bf16, isOutput=False)
    rm_d = nc.declare_dram_parameter("rmats", [128, 3, 2, 256], bf16, isOutput=False)
    out_d = nc.declare_dram_parameter(
        "out", [n_groups, 128, _G, 2, 256], bf16, isOutput=True
    )
    assert n_groups * _G == n_images

    with TileContext(nc) as tc:
        with (
            tc.tile_pool(name="consts", bufs=1) as cpool,
            tc.tile_pool(name="io", bufs=2) as io_pool,
            tc.tile_pool(name="tb", bufs=2) as tb_pool,
            tc.tile_pool(name="zsb", bufs=2) as z_pool,
            tc.tile_pool(name="ps", bufs=2, space="PSUM") as ps_pool,
        ):
            a0t = cpool.tile([128, 2, 256], bf16)
            a1t = cpool.tile([128, 2, 256], bf16)
            a2t = cpool.tile([128, 2, 256], bf16)
            rm = cpool.tile([128, 3, 2, 256], bf16)
            nc.scalar.dma_start(rm[:], rm_d[:])
            nc.scalar.dma_start(a0t[:], a0t_d[:])
            nc.scalar.dma_start(a1t[:], a1t_d[:])
            nc.scalar.dma_start(a2t[:], a2t_d[:])

            def stage_a(yh, yl, tb, i):
                """Col filters for image i -> z PSUM tile (transposed).

                z1/z2/z3 live in one 3-bank tile so a single Act copy
                casts all of them to SBUF bf16."""
                z = ps_pool.tile([128, 3, 2, 256], f32, tag="z")
                z1, z2, z3 = z[:, 0], z[:, 1], z[:, 2]
                for cc in range(2):
                    ws = slice(128 * cc, 128 * cc + 128)
                    js = slice(64 * cc, 64 * cc + 64)

                    def tbap(q, t):
                        # stationary [128, 64, 2] merges to one contiguous
                        # 128-wide free dim: c2 = 2w+ri
                        return tb[:, i, q, t, js, :]

                    # z1: lh pair (q=0, col A1) + Yl (col A0, banded split)
                    nc.tensor.matmul(
                        z1[:, cc, :], tbap(0, 0), rm[:, 0, 0, :],
                        start=True, stop=False,
                    )
                    nc.tensor.matmul(
                        z1[:, cc, :], tbap(0, 1), rm[:, 0, 1, :],
                        start=False, stop=False,
                    )
                    nc.tensor.matmul(
                        z1[:, cc, 0:136], yl[:, i, 0, ws], a0t[:, 0, 0:136],
                        start=False, stop=False,
                    )
                    nc.tensor.matmul(
                        z1[:, cc, 120:256], yl[:, i, 1, ws], a0t[:, 1, 120:256],
                        start=False, stop=True,
                    )
                    # z2: hl pair (q=2, col A0); row filter A1 later
                    nc.tensor.matmul(
                        z2[:, cc, :], tbap(2, 0), rm[:, 2, 0, :],
                        start=True, stop=False,
                    )
                    nc.tensor.matmul(
                        z2[:, cc, :], tbap(2, 1), rm[:, 2, 1, :],
                        start=False, stop=True,
                    )
                    # z3: hh pair (q=1, col A2); row filter A2 later
                    nc.tensor.matmul(
                        z3[:, cc, :], tbap(1, 0), rm[:, 1, 0, :],
                        start=True, stop=False,
                    )
                    nc.tensor.matmul(
                        z3[:, cc, :], tbap(1, 1), rm[:, 1, 1, :],
                        start=False, stop=True,
                    )
                # PSUM -> SBUF bf16 cast: one wide Act copy for all 3 paths
                zs = z_pool.tile([128, 3, 2, 256], bf16, tag="zs")
                nc.scalar.copy(zs[:], z[:])
                return zs

            def stage_b(zs, out_sb, g, i):
                """Row filters: y[r, c] = sum_paths Z^T @ A^T, banded."""
                z1s, z2s, z3s = zs[:, 0], zs[:, 1], zs[:, 2]
                yp = ps_pool.tile([128, 2, 256], f32, tag="yp")
                for r in range(2):
                    rs = slice(128 * r, 128 * r + 128)
                    # A1 path first: its k0 [0:144] starts the bank, k1 split
                    # [144:256]+[112:144] keeps every first-touch uniform.
                    nc.tensor.matmul(
                        yp[:, r, 0:144], z2s[:, 0, rs], a1t[:, 0, 0:144],
                        start=True, stop=False,
                    )
                    nc.tensor.matmul(
                        yp[:, r, 144:256], z2s[:, 1, rs], a1t[:, 1, 144:256],
                        start=False, stop=False,
                    )
                    nc.tensor.matmul(
                        yp[:, r, 112:144], z2s[:, 1, rs], a1t[:, 1, 112:144],
                        start=False, stop=False,
                    )
                    nc.tensor.matmul(
                        yp[:, r, 0:136], z1s[:, 0, rs], a0t[:, 0, 0:136],
                        start=False, stop=False,
                    )
                    nc.tensor.matmul(
                        yp[:, r, 120:256], z1s[:, 1, rs], a0t[:, 1, 120:256],
                        start=False, stop=False,
                    )
                    nc.tensor.matmul(
                        yp[:, r, 0:136], z3s[:, 0, rs], a2t[:, 0, 0:136],
                        start=False, stop=False,
                    )
                    nc.tensor.matmul(
                        yp[:, r, 120:256], z3s[:, 1, rs], a2t[:, 1, 120:256],
                        start=False, stop=True,
                    )
                if i % 2 == 0:
                    nc.scalar.copy(out_sb[:, i, :, :], yp[:])
                else:
                    nc.vector.tensor_copy(out_sb[:, i, :, :], yp[:])
                if i == _G - 1:
                    nc.gpsimd.dma_start(out_d[g], out_sb[:])

            # software pipeline: A(i+1) is issued before B(i) so the PE never
            # stalls on the PSUM->SBUF cast of z(i).
            pend = None  # (zs, out_sb, g, i)
            for g in range(n_groups):
                yh = io_pool.tile([128, 2, _G, 6, 128, 2], bf16, tag="yh", bufs=3)
                yl = io_pool.tile([128, _G, 2, 256], bf16, tag="yl")
                # S-variant and B-variant on separate queues so they overlap
                nc.sync.dma_start(yh[:, 0], yh_d[g, :, 0])
                nc.gpsimd.dma_start(yh[:, 1], yh_d[g, :, 1])
                nc.sync.dma_start(yl[:], yl_d[g])

                # tb[:, i, q, 0] = top, tb[:, i, q, 1] = bot -- all pairs in
                # three big contiguous ops (q=0 bot on Pool for balance).
                tb = tb_pool.tile([128, _G, 3, 2, 128, 2], bf16, tag="tb")
                nc.vector.tensor_add(
                    tb[:, :, :, 0, :, :], yh[:, 0, :, 0:3], yh[:, 0, :, 3:6]
                )
                nc.gpsimd.tensor_sub(
                    tb[:, :, 0, 1, :, :], yh[:, 1, :, 0], yh[:, 1, :, 3]
                )
                nc.vector.tensor_sub(
                    tb[:, :, 1:3, 1, :, :], yh[:, 1, :, 1:3], yh[:, 1, :, 4:6]
                )

                out_sb = io_pool.tile([128, _G, 2, 256], bf16, tag="out_sb")
                for i in range(_G):
                    zs = stage_a(yh, yl, tb, i)
                    if pend is not None:
                        stage_b(*pend)
                    pend = (zs, out_sb, g, i)
            stage_b(*pend)
    nc.compile()
    return nc


_NC_CACHE = {}


def _get_nc(n_images):
    if n_images not in _NC_CACHE:
        _NC_CACHE[n_images] = build_nc(n_images)
    return _NC_CACHE[n_images]


def pack_inputs(Yl_k, Yhr_k, Yhi_k):
    """Per-core repack into group-major bf16 layouts with contiguous rows.

    yhp[g, h, i, b, ri, w] = (Yhr|Yhi)[4g+i, b, h, w] -> 12KB/partition/group
    ylp[g, p, i, k, w] = Yl[4g+i, 128k+p, w]          ->  4KB/partition/group
    """
    ng = _C // _G
    perm = [0, 1, 2, 5, 4, 3]  # slot j pairs with slot 3+j
    r = Yhr_k.reshape(ng, _G, 6, 128, 128).transpose(0, 3, 1, 2, 4)[:, :, :, perm]
    im = Yhi_k.reshape(ng, _G, 6, 128, 128).transpose(0, 3, 1, 2, 4)[:, :, :, perm]
    yhp = np.empty((ng, 128, 2, _G, 6, 128, 2), dtype=BF)
    yhp[:, :, 0, :, :, :, 0] = r.astype(BF)   # S = (r | i)
    yhp[:, :, 0, :, :, :, 1] = im.astype(BF)
    yhp[:, :, 1, :, :, :, 0] = im.astype(BF)  # B = (i | -r)
    yhp[:, :, 1, :, :, :, 1] = (-r).astype(BF)
    ylp = np.ascontiguousarray(
        Yl_k.reshape(ng, _G, 2, 128, 256).transpose(0, 3, 1, 2, 4)
    ).astype(BF)
    return yhp, ylp


def unpack_output(outp):
    """outp (ng, 128, G, 2, 256) bf16: [g, p, i, k, w] = y[Gg+i, 128k+p, w]."""
    return np.ascontiguousarray(
        np.asarray(outp).transpose(0, 2, 3, 1, 4).reshape(-1, 256, 256)
    ).astype(np.float32)


def kernel(Yl, Yhr, Yhi, g0o, g1o, g2o):
    from concourse.bass_utils import run_bass_kernel_spmd

    Yl = np.asarray(Yl, dtype=np.float32)
    Yhr = np.asarray(Yhr, dtype=np.float32)
    Yhi = np.asarray(Yhi, dtype=np.float32)
    consts = build_consts(np.asarray(g0o), np.asarray(g1o), np.asarray(g2o))

    nc = _get_nc(_C)
    in_maps = []
    for k in range(_NCORES):
        yhp, ylp = pack_inputs(Yl[k], Yhr[k], Yhi[k])
        in_maps.append({"ylp": ylp, "yhp": yhp, **consts})
    res = run_bass_kernel_spmd(nc, in_maps, list(range(_NCORES)))
    out = np.stack([unpack_output(res.results[k]["out"]) for k in range(_NCORES)])
    return out.astype(np.float32)


# revision 35
# speedup vs baseline: 1.2271x; 1.2271x over previous
"""Inverse DTCWT (biort bandpass) level-1 reconstruction as a Bass/Tile kernel.

Math: the reference is
    y = (A0 @ Yl + A1 @ lh) @ A0^T + (A0 @ hl) @ A1^T + (A2 @ hh) @ A2^T
where A* are 256x256 banded matrices (1D taps + symmetric padding folded in)
and lh/hl/hh are the c2q quad-interleaves of subband pairs (0,5)/(2,3)/(1,4).

Row r of a c2q image comes from `top` (r even) or `bot` (r odd), each a
128x256 column-interleaved image; the column interleave is expressed in the
matmul stationary access pattern (w outer, ri inner) so the DVE builds the
four 128x128 combination blocks with plain contiguous adds/subs:
    top = [w1r+w2r | w1i+w2i]   bot = [w1i-w2i | w2r-w1r]
The row interleave never materializes: contraction over rows splits into
even/odd with host-precomputed matrices Re = A^T[0::2]/sqrt2, Ro = A^T[1::2]/sqrt2.

Stage A (col filters) runs with the *image tiles stationary* producing
transposed intermediates Z[c, h] in PSUM; stage B (row filters) consumes Z
slices as stationary against A^T and accumulates all three paths into one
PSUM bank in natural orientation. No transposes anywhere.

Everything runs in bfloat16 (I/O, matmul operands; PSUM accumulates fp32).
The A* matrices are banded (13/19/13 taps), so contraction chunks only
produce a ~136-144 wide band of output columns; matmul cost is (moving
free size) cycles, so every Yl / stage-B matmul streams only its band
instead of the full 256 columns (~25% fewer PE cycles).  PSUM start=True
marks the whole 2KB bank pending-zero, so partial-width accumulation is
exact as long as each byte's first touch comes from a uniformly-pending
write (write order below guarantees this).

Sharding: pure data parallel, batch dim (8) across 8 cores.
"""
import sys

if "/opt/trn_rl_repo" not in sys.path:
    sys.path.insert(0, "/opt/trn_rl_repo")

import ml_dtypes
import numpy as np

_C, _H = 64, 256  # channels per core, image size
_NCORES = 8
_G = 4  # images (channels) per group

BF = ml_dtypes.bfloat16


def _band_matrix(h, N):
    """A @ x == colfilter(x, h) with symmetric padding, in float64."""
    h = np.asarray(h, dtype=np.float64)
    L = h.shape[0]
    m = L // 2
    A = np.zeros((N, N), dtype=np.float64)
    for i in range(N):
        for k in range(L):
            s = i + k - m
            if s < 0:
                s = -1 - s
            elif s >= N:
                s = 2 * N - 1 - s
            A[i, s] += h[L - 1 - k]
    return A


def build_consts(g0o, g1o, g2o):
    """Host-side constant tensors handed to every core."""
    A0 = _band_matrix(g0o, _H).T  # stored transposed: [r, h]
    A1 = _band_matrix(g1o, _H).T
    A2 = _band_matrix(g2o, _H).T
    s2 = np.sqrt(2.0)

    def tile2(AT):  # [256, 256] -> [128, 2, 256] with [p, kr, h] = AT[128*kr+p, h]
        return np.ascontiguousarray(
            AT.reshape(2, 128, 256).transpose(1, 0, 2)
        ).astype(BF)

    a0t, a1t, a2t = tile2(A0), tile2(A1), tile2(A2)
    # rmats[q, e/o]: per-pair col-filter matrices; pair q uses bands (q, 5-q):
    #   q=0 (lh)   -> col filter A1 ; q=1 (hh) -> A2 ; q=2 (hl) -> A0
    rmats = np.stack(
        [
            np.stack([A1[0::2] / s2, A1[1::2] / s2]),
            np.stack([A2[0::2] / s2, A2[1::2] / s2]),
            np.stack([A0[0::2] / s2, A0[1::2] / s2]),
        ]
    )  # [3, 2, 128, 256]
    rm = np.ascontiguousarray(rmats.transpose(2, 0, 1, 3)).astype(BF)  # [128,3,2,256]
    return {"a0t": a0t, "a1t": a1t, "a2t": a2t, "rmats": rm}


def build_nc(n_images):
    import concourse.bacc as bacc
    import concourse.mybir as mybir
    from concourse.tile import TileContext

    f32 = mybir.dt.float32
    bf16 = mybir.dt.bfloat16
    nc = bacc.Bacc(None, target_bir_lowering=False, debug=False)

    n_groups = n_images // _G
    yl_d = nc.declare_dram_parameter(
        "ylp", [n_groups, 128, _G, 2, 256], bf16, isOutput=False
    )
    # yhp: host-interleaved bands S_b[hr, 2w+ri] = (r|i), in pair order
    # [0,1,2,5,4,3] so slot j pairs with slot 3+j and the c2q combinations
    # are three big ops per group: top = S_q + S_{5-q} (contiguous add),
    # bot even cols = i_q - i_{5q} and odd cols = r_{5q} - r_q (strided subs).
    yh_d = nc.declare_dram_parameter(
        "yhp", [n_groups, 128, _G, 6, 128, 2], bf16, isOutput=False
    )
    a0t_d = nc.declare_dram_parameter("a0t", [128, 2, 256], bf16, isOutput=False)
    a1t_d = nc.declare_dram_parameter("a1t", [128, 2, 256], bf16, isOutput=False)
    a2t_d = nc.declare_dram_parameter("a2t", [128, 2, 256], bf16, isOutput=False)
    rm_d = nc.declare_dram_parameter("rmats", [128, 3, 2, 256], bf16, isOutput=False)
    out_d = nc.declare_dram_parameter(
        "out", [n_groups, 128, _G, 2, 256], bf16, isOutput=True
    )
    assert n_groups * _G == n_images

    with TileContext(nc) as tc:
        with (
            tc.tile_pool(name="consts", bufs=1) as cpool,
            tc.tile_pool(name="io", bufs=2) as io_pool,
            tc.tile_pool(name="tb", bufs=2) as tb_pool,
            tc.tile_pool(name="zsb", bufs=2) as z_pool,
            tc.tile_pool(name="ps", bufs=2, space="PSUM") as ps_pool,
        ):
            a0t = cpool.tile([128, 2, 256], bf16)
            a1t = cpool.tile([128, 2, 256], bf16)
            a2t = cpool.tile([128, 2, 256], bf16)
            rm = cpool.tile([128, 3, 2, 256], bf16)
            nc.scalar.dma_start(rm[:], rm_d[:])
            nc.scalar.dma_start(a0t[:], a0t_d[:])
            nc.scalar.dma_start(a1t[:], a1t_d[:])
            nc.scalar.dma_start(a2t[:], a2t_d[:])

            def stage_a(yh, yl, tb, i):
                """Col filters for image i -> z PSUM tile (transposed).

                z1/z2/z3 live in one 3-bank tile so a single Act copy
                casts all of them to SBUF bf16."""
                z = ps_pool.tile([128, 3, 2, 256], f32, tag="z")
                # z2 in bank 0: stage B consumes it first (A1 path), so its
                # cast is issued as a separate leading op.
                z2, z1, z3 = z[:, 0], z[:, 1], z[:, 2]
                for cc in range(2):
                    ws = slice(128 * cc, 128 * cc + 128)
                    js = slice(64 * cc, 64 * cc + 64)

                    def tbap(q, t):
                        # stationary [128, 64, 2] merges to one contiguous
                        # 128-wide free dim: c2 = 2w+ri
                        return tb[:, i, q, t, js, :]

                    # z1: lh pair (q=0, col A1) + Yl (col A0, banded split)
                    nc.tensor.matmul(
                        z1[:, cc, :], tbap(0, 0), rm[:, 0, 0, :],
                        start=True, stop=False,
                    )
                    nc.tensor.matmul(
                        z1[:, cc, :], tbap(0, 1), rm[:, 0, 1, :],
                        start=False, stop=False,
                    )
                    nc.tensor.matmul(
                        z1[:, cc, 0:136], yl[:, i, 0, ws], a0t[:, 0, 0:136],
                        start=False, stop=False,
                    )
                    nc.tensor.matmul(
                        z1[:, cc, 120:256], yl[:, i, 1, ws], a0t[:, 1, 120:256],
                        start=False, stop=True,
                    )
                    # z2: hl pair (q=2, col A0); row filter A1 later
                    nc.tensor.matmul(
                        z2[:, cc, :], tbap(2, 0), rm[:, 2, 0, :],
                        start=True, stop=False,
                    )
                    nc.tensor.matmul(
                        z2[:, cc, :], tbap(2, 1), rm[:, 2, 1, :],
                        start=False, stop=True,
                    )
                    # z3: hh pair (q=1, col A2); row filter A2 later
                    nc.tensor.matmul(
                        z3[:, cc, :], tbap(1, 0), rm[:, 1, 0, :],
                        start=True, stop=False,
                    )
                    nc.tensor.matmul(
                        z3[:, cc, :], tbap(1, 1), rm[:, 1, 1, :],
                        start=False, stop=True,
                    )
                # PSUM -> SBUF bf16 cast: one wide Act copy (z2 in bank 0,
                # consumed first by stage B)
                zs = z_pool.tile([128, 3, 2, 256], bf16, tag="zs")
                nc.scalar.copy(zs[:], z[:])
                return zs

            def stage_b(zs, out_sb, g, i):
                """Row filters: y[r, c] = sum_paths Z^T @ A^T, banded."""
                z2s, z1s, z3s = zs[:, 0], zs[:, 1], zs[:, 2]
                yp = ps_pool.tile([128, 2, 256], f32, tag="yp")
                for r in range(2):
                    rs = slice(128 * r, 128 * r + 128)
                    # A1 path first: its k0 [0:144] starts the bank, k1 split
                    # [144:256]+[112:144] keeps every first-touch uniform.
                    nc.tensor.matmul(
                        yp[:, r, 0:144], z2s[:, 0, rs], a1t[:, 0, 0:144],
                        start=True, stop=False,
                    )
                    nc.tensor.matmul(
                        yp[:, r, 144:256], z2s[:, 1, rs], a1t[:, 1, 144:256],
                        start=False, stop=False,
                    )
                    nc.tensor.matmul(
                        yp[:, r, 112:144], z2s[:, 1, rs], a1t[:, 1, 112:144],
                        start=False, stop=False,
                    )
                    nc.tensor.matmul(
                        yp[:, r, 0:136], z1s[:, 0, rs], a0t[:, 0, 0:136],
                        start=False, stop=False,
                    )
                    nc.tensor.matmul(
                        yp[:, r, 120:256], z1s[:, 1, rs], a0t[:, 1, 120:256],
                        start=False, stop=False,
                    )
                    nc.tensor.matmul(
                        yp[:, r, 0:136], z3s[:, 0, rs], a2t[:, 0, 0:136],
                        start=False, stop=False,
                    )
                    nc.tensor.matmul(
                        yp[:, r, 120:256], z3s[:, 1, rs], a2t[:, 1, 120:256],
                        start=False, stop=True,
                    )
                if i % 2 == 0:
                    nc.scalar.copy(out_sb[:, i, :, :], yp[:])
                else:
                    nc.vector.tensor_copy(out_sb[:, i, :, :], yp[:])
                if i == _G - 1:
                    nc.gpsimd.dma_start(out_d[g], out_sb[:])

            # software pipeline: A(i+1) is issued before B(i) so the PE never
            # stalls on the PSUM->SBUF cast of z(i).
            pend = None  # (zs, out_sb, g, i)
            for g in range(n_groups):
                yh = io_pool.tile([128, _G, 6, 128, 2], bf16, tag="yh", bufs=3)
                yl = io_pool.tile([128, _G, 2, 256], bf16, tag="yl")
                nc.sync.dma_start(yh[:], yh_d[g])
                nc.gpsimd.dma_start(yl[:], yl_d[g])

                # tb[:, i, q, 0] = top, tb[:, i, q, 1] = bot -- all pairs in
                # three big batched ops (odd-col bot subs on Pool for balance).
                tb = tb_pool.tile([128, _G, 3, 2, 128, 2], bf16, tag="tb")
                nc.vector.tensor_add(
                    tb[:, :, :, 0, :, :], yh[:, :, 0:3], yh[:, :, 3:6]
                )
                nc.vector.tensor_sub(
                    tb[:, :, :, 1, :, 0], yh[:, :, 0:3, :, 1], yh[:, :, 3:6, :, 1]
                )
                nc.gpsimd.tensor_sub(
                    tb[:, :, :, 1, :, 1], yh[:, :, 3:6, :, 0], yh[:, :, 0:3, :, 0]
                )

                out_sb = io_pool.tile([128, _G, 2, 256], bf16, tag="out_sb")
                for i in range(_G):
                    zs = stage_a(yh, yl, tb, i)
                    if pend is not None:
                        stage_b(*pend)
                    pend = (zs, out_sb, g, i)
            stage_b(*pend)
    nc.compile()
    return nc


_NC_CACHE = {}


def _get_nc(n_images):
    if n_images not in _NC_CACHE:
        _NC_CACHE[n_images] = build_nc(n_images)
    return _NC_CACHE[n_images]


def pack_inputs(Yl_k, Yhr_k, Yhi_k):
    """Per-core repack into group-major bf16 layouts with contiguous rows.

    yhp[g, h, i, b, ri, w] = (Yhr|Yhi)[4g+i, b, h, w] -> 12KB/partition/group
    ylp[g, p, i, k, w] = Yl[4g+i, 128k+p, w]          ->  4KB/partition/group
    """
    ng = _C // _G
    perm = [0, 1, 2, 5, 4, 3]  # slot j pairs with slot 3+j
    r = Yhr_k.reshape(ng, _G, 6, 128, 128).transpose(0, 3, 1, 2, 4)[:, :, :, perm]
    im = Yhi_k.reshape(ng, _G, 6, 128, 128).transpose(0, 3, 1, 2, 4)[:, :, :, perm]
    yhp = np.empty((ng, 128, _G, 6, 128, 2), dtype=BF)
    yhp[:, :, :, :, :, 0] = r.astype(BF)   # S = (r | i)
    yhp[:, :, :, :, :, 1] = im.astype(BF)
    ylp = np.ascontiguousarray(
        Yl_k.reshape(ng, _G, 2, 128, 256).transpose(0, 3, 1, 2, 4)
    ).astype(BF)
    return yhp, ylp


def unpack_output(outp):
    """outp (ng, 128, G, 2, 256) bf16: [g, p, i, k, w] = y[Gg+i, 128k+p, w]."""
    return np.ascontiguousarray(
        np.asarray(outp).transpose(0, 2, 3, 1, 4).reshape(-1, 256, 256)
    ).astype(np.float32)


def kernel(Yl, Yhr, Yhi, g0o, g1o, g2o):
    from concourse.bass_utils import run_bass_kernel_spmd

    Yl = np.asarray(Yl, dtype=np.float32)
    Yhr = np.asarray(Yhr, dtype=np.float32)
    Yhi = np.asarray(Yhi, dtype=np.float32)
    consts = build_consts(np.asarray(g0o), np.asarray(g1o), np.asarray(g2o))

    nc = _get_nc(_C)
    in_maps = []
    for k in range(_NCORES):
        yhp, ylp = pack_inputs(Yl[k], Yhr[k], Yhi[k])
        in_maps.append({"ylp": ylp, "yhp": yhp, **consts})
    res = run_bass_kernel_spmd(nc, in_maps, list(range(_NCORES)))
    out = np.stack([unpack_output(res.results[k]["out"]) for k in range(_NCORES)])
    return out.astype(np.float32)


# revision 37
# speedup vs baseline: 1.3567x; 1.1056x over previous
"""Inverse DTCWT (biort bandpass) level-1 reconstruction as a Bass/Tile kernel.

Math: the reference is
    y = (A0 @ Yl + A1 @ lh) @ A0^T + (A0 @ hl) @ A1^T + (A2 @ hh) @ A2^T
where A* are 256x256 banded matrices (1D taps + symmetric padding folded in)
and lh/hl/hh are the c2q quad-interleaves of subband pairs (0,5)/(2,3)/(1,4).

Row r of a c2q image comes from `top` (r even) or `bot` (r odd), each a
128x256 column-interleaved image; the column interleave is expressed in the
matmul stationary access pattern (w outer, ri inner) so the DVE builds the
four 128x128 combination blocks with plain contiguous adds/subs:
    top = [w1r+w2r | w1i+w2i]   bot = [w1i-w2i | w2r-w1r]
The row interleave never materializes: contraction over rows splits into
even/odd with host-precomputed matrices Re = A^T[0::2]/sqrt2, Ro = A^T[1::2]/sqrt2.

Stage A (col filters) runs with the *image tiles stationary* producing
transposed intermediates Z[c, h] in PSUM; stage B (row filters) consumes Z
slices as stationary against A^T and accumulates all three paths into one
PSUM bank in natural orientation. No transposes anywhere.

Everything runs in bfloat16 (I/O, matmul operands; PSUM accumulates fp32).
The A* matrices are banded (13/19/13 taps), so contraction chunks only
produce a ~136-144 wide band of output columns; matmul cost is (moving
free size) cycles, so every Yl / stage-B matmul streams only its band
instead of the full 256 columns (~25% fewer PE cycles).  PSUM start=True
marks the whole 2KB bank pending-zero, so partial-width accumulation is
exact as long as each byte's first touch comes from a uniformly-pending
write (write order below guarantees this).

Sharding: pure data parallel, batch dim (8) across 8 cores.
"""
import sys

if "/opt/trn_rl_repo" not in sys.path:
    sys.path.insert(0, "/opt/trn_rl_repo")

import ml_dtypes
import numpy as np

_C, _H = 64, 256  # channels per core, image size
_NCORES = 8
_G = 4  # images (channels) per group

BF = ml_dtypes.bfloat16


def _band_matrix(h, N):
    """A @ x == colfilter(x, h) with symmetric padding, in float64."""
    h = np.asarray(h, dtype=np.float64)
    L = h.shape[0]
    m = L // 2
    A = np.zeros((N, N), dtype=np.float64)
    for i in range(N):
        for k in range(L):
            s = i + k - m
            if s < 0:
                s = -1 - s
            elif s >= N:
                s = 2 * N - 1 - s
            A[i, s] += h[L - 1 - k]
    return A


def build_consts(g0o, g1o, g2o):
    """Host-side constant tensors handed to every core."""
    A0 = _band_matrix(g0o, _H).T  # stored transposed: [r, h]
    A1 = _band_matrix(g1o, _H).T
    A2 = _band_matrix(g2o, _H).T
    s2 = np.sqrt(2.0)

    def tile2(AT):  # [256, 256] -> [128, 2, 256] with [p, kr, h] = AT[128*kr+p, h]
        return np.ascontiguousarray(
            AT.reshape(2, 128, 256).transpose(1, 0, 2)
        ).astype(BF)

    a0t, a1t, a2t = tile2(A0), tile2(A1), tile2(A2)
    # rmats[q, e/o]: per-pair col-filter matrices; pair q uses bands (q, 5-q):
    #   q=0 (lh)   -> col filter A1 ; q=1 (hh) -> A2 ; q=2 (hl) -> A0
    rmats = np.stack(
        [
            np.stack([A1[0::2] / s2, A1[1::2] / s2]),
            np.stack([A2[0::2] / s2, A2[1::2] / s2]),
            np.stack([A0[0::2] / s2, A0[1::2] / s2]),
        ]
    )  # [3, 2, 128, 256]
    rm = np.ascontiguousarray(rmats.transpose(2, 0, 1, 3)).astype(BF)  # [128,3,2,256]
    return {"a0t": a0t, "a1t": a1t, "a2t": a2t, "rmats": rm}


def build_nc(n_images):
    import concourse.bacc as bacc
    import concourse.mybir as mybir
    from concourse.tile import TileContext

    f32 = mybir.dt.float32
    bf16 = mybir.dt.bfloat16
    nc = bacc.Bacc(None, target_bir_lowering=False, debug=False)

    n_groups = n_images // _G
    yl_d = nc.declare_dram_parameter(
        "ylp", [n_groups, 128, _G, 2, 256], bf16, isOutput=False
    )
    # yhp: host-interleaved bands S_b[hr, 2w+ri] = (r|i), in pair order
    # [0,1,2,5,4,3] so slot j pairs with slot 3+j and the c2q combinations
    # are three big ops per group: top = S_q + S_{5-q} (contiguous add),
    # bot even cols = i_q - i_{5q} and odd cols = r_{5q} - r_q (strided subs).
    yh_d = nc.declare_dram_parameter(
        "yhp", [n_groups, 128, _G, 6, 128, 2], bf16, isOutput=False
    )
    a0t_d = nc.declare_dram_parameter("a0t", [128, 2, 256], bf16, isOutput=False)
    a1t_d = nc.declare_dram_parameter("a1t", [128, 2, 256], bf16, isOutput=False)
    a2t_d = nc.declare_dram_parameter("a2t", [128, 2, 256], bf16, isOutput=False)
    rm_d = nc.declare_dram_parameter("rmats", [128, 3, 2, 256], bf16, isOutput=False)
    out_d = nc.declare_dram_parameter(
        "out", [n_groups, 128, _G, 2, 256], bf16, isOutput=True
    )
    assert n_groups * _G == n_images

    with TileContext(nc) as tc:
        with (
            tc.tile_pool(name="consts", bufs=1) as cpool,
            tc.tile_pool(name="io", bufs=2) as io_pool,
            tc.tile_pool(name="tb", bufs=2) as tb_pool,
            tc.tile_pool(name="zsb", bufs=2) as z_pool,
            tc.tile_pool(name="ps", bufs=2, space="PSUM") as ps_pool,
        ):
            a0t = cpool.tile([128, 2, 256], bf16)
            a1t = cpool.tile([128, 2, 256], bf16)
            a2t = cpool.tile([128, 2, 256], bf16)
            rm = cpool.tile([128, 3, 2, 256], bf16)
            nc.scalar.dma_start(rm[:], rm_d[:])
            nc.scalar.dma_start(a0t[:], a0t_d[:])
            nc.scalar.dma_start(a1t[:], a1t_d[:])
            nc.scalar.dma_start(a2t[:], a2t_d[:])

            def stage_a(yh, yl, tb, i):
                """Col filters for image i -> z PSUM tile (transposed).

                z1/z2/z3 live in one 3-bank tile so a single Act copy
                casts all of them to SBUF bf16."""
                z = ps_pool.tile([128, 3, 2, 256], f32, tag="z")
                # z2 in bank 0: stage B consumes it first (A1 path), so its
                # cast is issued as a separate leading op.
                z2, z1, z3 = z[:, 0], z[:, 1], z[:, 2]
                for cc in range(2):
                    ws = slice(128 * cc, 128 * cc + 128)
                    js = slice(64 * cc, 64 * cc + 64)

                    def tbap(q, t):
                        # stationary [128, 64, 2] merges to one contiguous
                        # 128-wide free dim: c2 = 2w+ri
                        return tb[:, i, q, t, js, :]

                    # z1: lh pair (q=0, col A1) + Yl (col A0, banded split)
                    nc.tensor.matmul(
                        z1[:, cc, :], tbap(0, 0), rm[:, 0, 0, :],
                        start=True, stop=False,
                    )
                    nc.tensor.matmul(
                        z1[:, cc, :], tbap(0, 1), rm[:, 0, 1, :],
                        start=False, stop=False,
                    )
                    nc.tensor.matmul(
                        z1[:, cc, 0:136], yl[:, i, 0, ws], a0t[:, 0, 0:136],
                        start=False, stop=False,
                    )
                    nc.tensor.matmul(
                        z1[:, cc, 120:256], yl[:, i, 1, ws], a0t[:, 1, 120:256],
                        start=False, stop=True,
                    )
                    # z2: hl pair (q=2, col A0); row filter A1 later
                    nc.tensor.matmul(
                        z2[:, cc, :], tbap(2, 0), rm[:, 2, 0, :],
                        start=True, stop=False,
                    )
                    nc.tensor.matmul(
                        z2[:, cc, :], tbap(2, 1), rm[:, 2, 1, :],
                        start=False, stop=True,
                    )
                    # z3: hh pair (q=1, col A2); row filter A2 later
                    nc.tensor.matmul(
                        z3[:, cc, :], tbap(1, 0), rm[:, 1, 0, :],
                        start=True, stop=False,
                    )
                    nc.tensor.matmul(
                        z3[:, cc, :], tbap(1, 1), rm[:, 1, 1, :],
                        start=False, stop=True,
                    )
                # PSUM -> SBUF bf16 cast: one wide Act copy (z2 in bank 0,
                # consumed first by stage B)
                zs = z_pool.tile([128, 3, 2, 256], bf16, tag="zs")
                nc.scalar.copy(zs[:], z[:])
                return zs

            def stage_b(zs, out_sb, g, i):
                """Row filters: y[r, c] = sum_paths Z^T @ A^T, banded."""
                z2s, z1s, z3s = zs[:, 0], zs[:, 1], zs[:, 2]
                yp = ps_pool.tile([128, 2, 256], f32, tag="yp")
                for r in range(2):
                    rs = slice(128 * r, 128 * r + 128)
                    # A1 path first: its k0 [0:144] starts the bank, k1 split
                    # [144:256]+[112:144] keeps every first-touch uniform.
                    nc.tensor.matmul(
                        yp[:, r, 0:144], z2s[:, 0, rs], a1t[:, 0, 0:144],
                        start=True, stop=False,
                    )
                    nc.tensor.matmul(
                        yp[:, r, 144:256], z2s[:, 1, rs], a1t[:, 1, 144:256],
                        start=False, stop=False,
                    )
                    nc.tensor.matmul(
                        yp[:, r, 112:144], z2s[:, 1, rs], a1t[:, 1, 112:144],
                        start=False, stop=False,
                    )
                    nc.tensor.matmul(
                        yp[:, r, 0:136], z1s[:, 0, rs], a0t[:, 0, 0:136],
                        start=False, stop=False,
                    )
                    nc.tensor.matmul(
                        yp[:, r, 120:256], z1s[:, 1, rs], a0t[:, 1, 120:256],
                        start=False, stop=False,
                    )
                    nc.tensor.matmul(
                        yp[:, r, 0:136], z3s[:, 0, rs], a2t[:, 0, 0:136],
                        start=False, stop=False,
                    )
                    nc.tensor.matmul(
                        yp[:, r, 120:256], z3s[:, 1, rs], a2t[:, 1, 120:256],
                        start=False, stop=True,
                    )
                if i % 2 == 0:
                    nc.vector.tensor_copy(out_sb[:, i, :, :], yp[:])
                else:
                    nc.scalar.copy(out_sb[:, i, :, :], yp[:])
                if i == _G - 1:
                    nc.gpsimd.dma_start(out_d[g], out_sb[:])

            # software pipeline: A(i+1) is issued before B(i) so the PE never
            # stalls on the PSUM->SBUF cast of z(i).
            pend = None  # (zs, out_sb, g, i)
            for g in range(n_groups):
                yh = io_pool.tile([128, _G, 6, 128, 2], bf16, tag="yh", bufs=3)
                yl = io_pool.tile([128, _G, 2, 256], bf16, tag="yl")
                nc.sync.dma_start(yh[:], yh_d[g])
                nc.sync.dma_start(yl[:], yl_d[g])

                # tb[:, i, q, 0] = top = S_q + S_{5-q} (contiguous add);
                # tb[:, i, q, 1] = bot: even cols i_q-i_{5q}, odd cols
                # r_{5q}-r_q (pair-swapped strided subs; odd subs on Pool).
                tb = tb_pool.tile([128, _G, 3, 2, 128, 2], bf16, tag="tb")
                for q in range(3):
                    nc.vector.tensor_add(
                        tb[:, :, q, 0, :, :], yh[:, :, q], yh[:, :, q + 3]
                    )
                    nc.vector.tensor_sub(
                        tb[:, :, q, 1, :, 0], yh[:, :, q, :, 1], yh[:, :, q + 3, :, 1]
                    )
                    nc.gpsimd.tensor_sub(
                        tb[:, :, q, 1, :, 1], yh[:, :, q + 3, :, 0], yh[:, :, q, :, 0]
                    )

                out_sb = io_pool.tile([128, _G, 2, 256], bf16, tag="out_sb")
                for i in range(_G):
                    zs = stage_a(yh, yl, tb, i)
                    if pend is not None:
                        stage_b(*pend)
                    pend = (zs, out_sb, g, i)
            stage_b(*pend)
    nc.compile()
    return nc


_NC_CACHE = {}


def _get_nc(n_images):
    if n_images not in _NC_CACHE:
        _NC_CACHE[n_images] = build_nc(n_images)
    return _NC_CACHE[n_images]


def pack_inputs(Yl_k, Yhr_k, Yhi_k):
    """Per-core repack into group-major bf16 layouts with contiguous rows.

    yhp[g, h, i, b, ri, w] = (Yhr|Yhi)[4g+i, b, h, w] -> 12KB/partition/group
    ylp[g, p, i, k, w] = Yl[4g+i, 128k+p, w]          ->  4KB/partition/group
    """
    ng = _C // _G
    perm = [0, 1, 2, 5, 4, 3]  # slot j pairs with slot 3+j
    r = Yhr_k.reshape(ng, _G, 6, 128, 128).transpose(0, 3, 1, 2, 4)[:, :, :, perm]
    im = Yhi_k.reshape(ng, _G, 6, 128, 128).transpose(0, 3, 1, 2, 4)[:, :, :, perm]
    yhp = np.empty((ng, 128, _G, 6, 128, 2), dtype=BF)
    yhp[:, :, :, :, :, 0] = r.astype(BF)   # S = (r | i)
    yhp[:, :, :, :, :, 1] = im.astype(BF)
    ylp = np.ascontiguousarray(
        Yl_k.reshape(ng, _G, 2, 128, 256).transpose(0, 3, 1, 2, 4)
    ).astype(BF)
    return yhp, ylp


def unpack_output(outp):
    """outp (ng, 128, G, 2, 256) bf16: [g, p, i, k, w] = y[Gg+i, 128k+p, w]."""
    return np.ascontiguousarray(
        np.asarray(outp).transpose(0, 2, 3, 1, 4).reshape(-1, 256, 256)
    ).astype(np.float32)


def kernel(Yl, Yhr, Yhi, g0o, g1o, g2o):
    from concourse.bass_utils import run_bass_kernel_spmd

    Yl = np.asarray(Yl, dtype=np.float32)
    Yhr = np.asarray(Yhr, dtype=np.float32)
    Yhi = np.asarray(Yhi, dtype=np.float32)
    consts = build_consts(np.asarray(g0o), np.asarray(g1o), np.asarray(g2o))

    nc = _get_nc(_C)
    in_maps = []
    for k in range(_NCORES):
        yhp, ylp = pack_inputs(Yl[k], Yhr[k], Yhi[k])
        in_maps.append({"ylp": ylp, "yhp": yhp, **consts})
    res = run_bass_kernel_spmd(nc, in_maps, list(range(_NCORES)))
    out = np.stack([unpack_output(res.results[k]["out"]) for k in range(_NCORES)])
    return out.astype(np.float32)


# revision 39
# speedup vs baseline: 1.4110x; 1.0400x over previous
"""Inverse DTCWT (biort bandpass) level-1 reconstruction as a Bass/Tile kernel.

Math: the reference is
    y = (A0 @ Yl + A1 @ lh) @ A0^T + (A0 @ hl) @ A1^T + (A2 @ hh) @ A2^T
where A* are 256x256 banded matrices (1D taps + symmetric padding folded in)
and lh/hl/hh are the c2q quad-interleaves of subband pairs (0,5)/(2,3)/(1,4).

Row r of a c2q image comes from `top` (r even) or `bot` (r odd), each a
128x256 column-interleaved image; the column interleave is expressed in the
matmul stationary access pattern (w outer, ri inner) so the DVE builds the
four 128x128 combination blocks with plain contiguous adds/subs:
    top = [w1r+w2r | w1i+w2i]   bot = [w1i-w2i | w2r-w1r]
The row interleave never materializes: contraction over rows splits into
even/odd with host-precomputed matrices Re = A^T[0::2]/sqrt2, Ro = A^T[1::2]/sqrt2.

Stage A (col filters) runs with the *image tiles stationary* producing
transposed intermediates Z[c, h] in PSUM; stage B (row filters) consumes Z
slices as stationary against A^T and accumulates all three paths into one
PSUM bank in natural orientation. No transposes anywhere.

Everything runs in bfloat16 (I/O, matmul operands; PSUM accumulates fp32).
The A* matrices are banded (13/19/13 taps), so contraction chunks only
produce a ~136-144 wide band of output columns; matmul cost is (moving
free size) cycles, so every Yl / stage-B matmul streams only its band
instead of the full 256 columns (~25% fewer PE cycles).  PSUM start=True
marks the whole 2KB bank pending-zero, so partial-width accumulation is
exact as long as each byte's first touch comes from a uniformly-pending
write (write order below guarantees this).

Sharding: pure data parallel, batch dim (8) across 8 cores.
"""
import sys

if "/opt/trn_rl_repo" not in sys.path:
    sys.path.insert(0, "/opt/trn_rl_repo")

import ml_dtypes
import numpy as np

_C, _H = 64, 256  # channels per core, image size
_NCORES = 8
_G = 4  # images (channels) per group

BF = ml_dtypes.bfloat16


def _band_matrix(h, N):
    """A @ x == colfilter(x, h) with symmetric padding, in float64."""
    h = np.asarray(h, dtype=np.float64)
    L = h.shape[0]
    m = L // 2
    A = np.zeros((N, N), dtype=np.float64)
    for i in range(N):
        for k in range(L):
            s = i + k - m
            if s < 0:
                s = -1 - s
            elif s >= N:
                s = 2 * N - 1 - s
            A[i, s] += h[L - 1 - k]
    return A


def build_consts(g0o, g1o, g2o):
    """Host-side constant tensors handed to every core."""
    A0 = _band_matrix(g0o, _H).T  # stored transposed: [r, h]
    A1 = _band_matrix(g1o, _H).T
    A2 = _band_matrix(g2o, _H).T
    s2 = np.sqrt(2.0)

    def tile2(AT):  # [256, 256] -> [128, 2, 256] with [p, kr, h] = AT[128*kr+p, h]
        return np.ascontiguousarray(
            AT.reshape(2, 128, 256).transpose(1, 0, 2)
        ).astype(BF)

    a0t, a1t, a2t = tile2(A0), tile2(A1), tile2(A2)
    # rmats[q, e/o]: per-pair col-filter matrices; pair q uses bands (q, 5-q):
    #   q=0 (lh)   -> col filter A1 ; q=1 (hh) -> A2 ; q=2 (hl) -> A0
    rmats = np.stack(
        [
            np.stack([A1[0::2] / s2, A1[1::2] / s2]),
            np.stack([A2[0::2] / s2, A2[1::2] / s2]),
            np.stack([A0[0::2] / s2, A0[1::2] / s2]),
        ]
    )  # [3, 2, 128, 256]
    rm = np.ascontiguousarray(rmats.transpose(2, 0, 1, 3)).astype(BF)  # [128,3,2,256]
    return {"a0t": a0t, "a1t": a1t, "a2t": a2t, "rmats": rm}


def build_nc(n_images):
    import concourse.bacc as bacc
    import concourse.mybir as mybir
    from concourse.tile import TileContext

    f32 = mybir.dt.float32
    bf16 = mybir.dt.bfloat16
    nc = bacc.Bacc(None, target_bir_lowering=False, debug=False)

    n_groups = n_images // _G
    yl_d = nc.declare_dram_parameter(
        "ylp", [n_groups, 128, _G, 2, 256], bf16, isOutput=False
    )
    # yhp: host-interleaved bands S_b[hr, 2w+ri] = (r|i), in pair order
    # [0,1,2,5,4,3] so slot j pairs with slot 3+j and the c2q combinations
    # are three big ops per group: top = S_q + S_{5-q} (contiguous add),
    # bot even cols = i_q - i_{5q} and odd cols = r_{5q} - r_q (strided subs).
    yh_d = nc.declare_dram_parameter(
        "yhp", [n_groups, 128, _G, 6, 128, 2], bf16, isOutput=False
    )
    a0t_d = nc.declare_dram_parameter("a0t", [128, 2, 256], bf16, isOutput=False)
    a1t_d = nc.declare_dram_parameter("a1t", [128, 2, 256], bf16, isOutput=False)
    a2t_d = nc.declare_dram_parameter("a2t", [128, 2, 256], bf16, isOutput=False)
    rm_d = nc.declare_dram_parameter("rmats", [128, 3, 2, 256], bf16, isOutput=False)
    out_d = nc.declare_dram_parameter(
        "out", [n_groups, 128, _G, 2, 256], bf16, isOutput=True
    )
    assert n_groups * _G == n_images

    with TileContext(nc) as tc:
        with (
            tc.tile_pool(name="consts", bufs=1) as cpool,
            tc.tile_pool(name="io", bufs=2) as io_pool,
            tc.tile_pool(name="tb", bufs=2) as tb_pool,
            tc.tile_pool(name="zsb", bufs=2) as z_pool,
            tc.tile_pool(name="ps", bufs=2, space="PSUM") as ps_pool,
        ):
            a0t = cpool.tile([128, 2, 256], bf16)
            a1t = cpool.tile([128, 2, 256], bf16)
            a2t = cpool.tile([128, 2, 256], bf16)
            rm = cpool.tile([128, 3, 2, 256], bf16)
            nc.scalar.dma_start(rm[:], rm_d[:])
            nc.scalar.dma_start(a0t[:], a0t_d[:])
            nc.scalar.dma_start(a1t[:], a1t_d[:])
            nc.scalar.dma_start(a2t[:], a2t_d[:])

            def stage_a(yh, yl, tb, i):
                """Col filters for image i -> z PSUM tile (transposed).

                z1/z2/z3 live in one 3-bank tile so a single Act copy
                casts all of them to SBUF bf16."""
                z = ps_pool.tile([128, 3, 2, 256], f32, tag="z")
                # z2 in bank 0: stage B consumes it first (A1 path), so its
                # cast is issued as a separate leading op.
                z2, z1, z3 = z[:, 0], z[:, 1], z[:, 2]
                for cc in range(2):
                    ws = slice(128 * cc, 128 * cc + 128)
                    js = slice(64 * cc, 64 * cc + 64)

                    def tbap(q, t):
                        # stationary [128, 64, 2] merges to one contiguous
                        # 128-wide free dim: c2 = 2w+ri
                        return tb[:, i, q, t, js, :]

                    # z1: lh pair (q=0, col A1) + Yl (col A0, banded split)
                    nc.tensor.matmul(
                        z1[:, cc, :], tbap(0, 0), rm[:, 0, 0, :],
                        start=True, stop=False,
                    )
                    nc.tensor.matmul(
                        z1[:, cc, :], tbap(0, 1), rm[:, 0, 1, :],
                        start=False, stop=False,
                    )
                    nc.tensor.matmul(
                        z1[:, cc, 0:136], yl[:, i, 0, ws], a0t[:, 0, 0:136],
                        start=False, stop=False,
                    )
                    nc.tensor.matmul(
                        z1[:, cc, 120:256], yl[:, i, 1, ws], a0t[:, 1, 120:256],
                        start=False, stop=True,
                    )
                    # z2: hl pair (q=2, col A0); row filter A1 later
                    nc.tensor.matmul(
                        z2[:, cc, :], tbap(2, 0), rm[:, 2, 0, :],
                        start=True, stop=False,
                    )
                    nc.tensor.matmul(
                        z2[:, cc, :], tbap(2, 1), rm[:, 2, 1, :],
                        start=False, stop=True,
                    )
                    # z3: hh pair (q=1, col A2); row filter A2 later
                    nc.tensor.matmul(
                        z3[:, cc, :], tbap(1, 0), rm[:, 1, 0, :],
                        start=True, stop=False,
                    )
                    nc.tensor.matmul(
                        z3[:, cc, :], tbap(1, 1), rm[:, 1, 1, :],
                        start=False, stop=True,
                    )
                # PSUM -> SBUF bf16 cast: one wide Act copy (z2 in bank 0,
                # consumed first by stage B)
                zs = z_pool.tile([128, 3, 2, 256], bf16, tag="zs")
                nc.scalar.copy(zs[:], z[:])
                return zs

            def stage_b(zs, out_sb, g, i):
                """Row filters: y[r, c] = sum_paths Z^T @ A^T, banded."""
                z2s, z1s, z3s = zs[:, 0], zs[:, 1], zs[:, 2]
                yp = ps_pool.tile([128, 2, 256], f32, tag="yp")
                for r in range(2):
                    rs = slice(128 * r, 128 * r + 128)
                    # A1 path first: its k0 [0:144] starts the bank, k1 split
                    # [144:256]+[112:144] keeps every first-touch uniform.
                    nc.tensor.matmul(
                        yp[:, r, 0:144], z2s[:, 0, rs], a1t[:, 0, 0:144],
                        start=True, stop=False,
                    )
                    nc.tensor.matmul(
                        yp[:, r, 144:256], z2s[:, 1, rs], a1t[:, 1, 144:256],
                        start=False, stop=False,
                    )
                    nc.tensor.matmul(
                        yp[:, r, 112:144], z2s[:, 1, rs], a1t[:, 1, 112:144],
                        start=False, stop=False,
                    )
                    nc.tensor.matmul(
                        yp[:, r, 0:136], z1s[:, 0, rs], a0t[:, 0, 0:136],
                        start=False, stop=False,
                    )
                    nc.tensor.matmul(
                        yp[:, r, 120:256], z1s[:, 1, rs], a0t[:, 1, 120:256],
                        start=False, stop=False,
                    )
                    nc.tensor.matmul(
                        yp[:, r, 0:136], z3s[:, 0, rs], a2t[:, 0, 0:136],
                        start=False, stop=False,
                    )
                    nc.tensor.matmul(
                        yp[:, r, 120:256], z3s[:, 1, rs], a2t[:, 1, 120:256],
                        start=False, stop=True,
                    )
                if i % 2 == 0:
                    nc.vector.tensor_copy(out_sb[:, i, :, :], yp[:])
                else:
                    nc.scalar.copy(out_sb[:, i, :, :], yp[:])
                if g == n_groups - 1:
                    # epilogue: drain the output per image, not per group
                    nc.gpsimd.dma_start(out_d[g, :, i], out_sb[:, i])
                elif i == _G - 1:
                    nc.gpsimd.dma_start(out_d[g], out_sb[:])

            # software pipeline: A(i+1) is issued before B(i) so the PE never
            # stalls on the PSUM->SBUF cast of z(i).
            pend = None  # (zs, out_sb, g, i)
            for g in range(n_groups):
                yh = io_pool.tile([128, _G, 6, 128, 2], bf16, tag="yh", bufs=3)
                yl = io_pool.tile([128, _G, 2, 256], bf16, tag="yl")
                # tb[:, i, q, 0] = top = S_q + S_{5-q} (contiguous add);
                # tb[:, i, q, 1] = bot: even cols i_q-i_{5q}, odd cols
                # r_{5q}-r_q (pair-swapped strided subs; odd subs on Pool).
                tb = tb_pool.tile([128, _G, 3, 2, 128, 2], bf16, tag="tb")
                if g == 0:
                    # prologue: per-image DMA + c2q so the PE starts ASAP
                    nc.sync.dma_start(yh[:, 0], yh_d[g, :, 0])
                    nc.sync.dma_start(yl[:], yl_d[g])
                    for i in range(1, _G):
                        nc.sync.dma_start(yh[:, i], yh_d[g, :, i])
                    for i in range(_G):
                        ii = slice(i, i + 1)
                        for q in range(3):
                            nc.vector.tensor_add(
                                tb[:, ii, q, 0, :, :], yh[:, ii, q], yh[:, ii, q + 3]
                            )
                            nc.vector.tensor_sub(
                                tb[:, ii, q, 1, :, 0],
                                yh[:, ii, q, :, 1], yh[:, ii, q + 3, :, 1],
                            )
                            nc.gpsimd.tensor_sub(
                                tb[:, ii, q, 1, :, 1],
                                yh[:, ii, q + 3, :, 0], yh[:, ii, q, :, 0],
                            )
                else:
                    nc.sync.dma_start(yh[:], yh_d[g])
                    nc.sync.dma_start(yl[:], yl_d[g])
                    for q in range(3):
                        nc.vector.tensor_add(
                            tb[:, :, q, 0, :, :], yh[:, :, q], yh[:, :, q + 3]
                        )
                        nc.vector.tensor_sub(
                            tb[:, :, q, 1, :, 0], yh[:, :, q, :, 1], yh[:, :, q + 3, :, 1]
                        )
                        nc.gpsimd.tensor_sub(
                            tb[:, :, q, 1, :, 1], yh[:, :, q + 3, :, 0], yh[:, :, q, :, 0]
                        )

                out_sb = io_pool.tile([128, _G, 2, 256], bf16, tag="out_sb")
                for i in range(_G):
                    zs = stage_a(yh, yl, tb, i)
                    if pend is not None:
                        stage_b(*pend)
                    pend = (zs, out_sb, g, i)
            stage_b(*pend)
    nc.compile()
    return nc


_NC_CACHE = {}


def _get_nc(n_images):
    if n_images not in _NC_CACHE:
        _NC_CACHE[n_images] = build_nc(n_images)
    return _NC_CACHE[n_images]


def pack_inputs(Yl_k, Yhr_k, Yhi_k):
    """Per-core repack into group-major bf16 layouts with contiguous rows.

    yhp[g, h, i, b, ri, w] = (Yhr|Yhi)[4g+i, b, h, w] -> 12KB/partition/group
    ylp[g, p, i, k, w] = Yl[4g+i, 128k+p, w]          ->  4KB/partition/group
    """
    ng = _C // _G
    perm = [0, 1, 2, 5, 4, 3]  # slot j pairs with slot 3+j
    r = Yhr_k.reshape(ng, _G, 6, 128, 128).transpose(0, 3, 1, 2, 4)[:, :, :, perm]
    im = Yhi_k.reshape(ng, _G, 6, 128, 128).transpose(0, 3, 1, 2, 4)[:, :, :, perm]
    yhp = np.empty((ng, 128, _G, 6, 128, 2), dtype=BF)
    yhp[:, :, :, :, :, 0] = r.astype(BF)   # S = (r | i)
    yhp[:, :, :, :, :, 1] = im.astype(BF)
    ylp = np.ascontiguousarray(
        Yl_k.reshape(ng, _G, 2, 128, 256).transpose(0, 3, 1, 2, 4)
    ).astype(BF)
    return yhp, ylp


def unpack_output(outp):
    """outp (ng, 128, G, 2, 256) bf16: [g, p, i, k, w] = y[Gg+i, 128k+p, w]."""
    return np.ascontiguousarray(
        np.asarray(outp).transpose(0, 2, 3, 1, 4).reshape(-1, 256, 256)
    ).astype(np.float32)


def kernel(Yl, Yhr, Yhi, g0o, g1o, g2o):
    from concourse.bass_utils import run_bass_kernel_spmd

    Yl = np.asarray(Yl, dtype=np.float32)
    Yhr = np.asarray(Yhr, dtype=np.float32)
    Yhi = np.asarray(Yhi, dtype=np.float32)
    consts = build_consts(np.asarray(g0o), np.asarray(g1o), np.asarray(g2o))

    nc = _get_nc(_C)
    in_maps = []
    for k in range(_NCORES):
        yhp, ylp = pack_inputs(Yl[k], Yhr[k], Yhi[k])
        in_maps.append({"ylp": ylp, "yhp": yhp, **consts})
    res = run_bass_kernel_spmd(nc, in_maps, list(range(_NCORES)))
    out = np.stack([unpack_output(res.results[k]["out"]) for k in range(_NCORES)])
    return out.astype(np.float32)
